# revision 2
# baseline (speedup 1.0000x reference)
"""Trainium2 Bass kernel for nn_MultiHeadMLPAttentionModel.

Model: per (b, n) point: pairwise = [radar_b(4), pt(2)] (radar constant over n).
  h1 = relu(pairwise @ enc_w1 + enc_b1)            [B,N,64]
  pf = h1 @ enc_w2 + enc_b2                        [B,N,64]
  sh = relu(einsum('bnf,hfd', pairwise, sc_w1) + sc_b1)
  logits = einsum('bnhd,hd', sh, sc_w2) + sc_b2    [B,N,4]
  w = softmax(logits, axis=n)
  ctx = einsum('bnh,bnd', w, pf)  -> out MLP -> [B]

Key algebraic restructurings used here:
  * pooling commutes with the (linear) second encoder layer since softmax
    weights sum to 1:  ctx = (sum_n w * h1) @ enc_w2 + enc_b2.  This removes
    the N-scale enc2 matmul entirely.
  * sc_b2 is constant over n, so it drops out of the softmax.
  * the radar part of pairwise is constant over n, so all layer-1 radar
    contributions fold into per-b bias vectors (computed on host: ~200 KFLOP
    of the model's 13 GFLOP).
  * softmax is computed without max-subtraction (logits are O(1) for this
    model; exp is evaluated in fp32) and normalization is deferred: the
    pooling matmul accumulates unnormalized sum_n exp(l)*h1 plus sum_n exp(l)
    (via an appended ones column), and the division happens once per b.

Sharding: pure data parallel over B: 8 cores x 16 rows each.  One SPMD Bass
program; per-core inputs differ only in data.
"""

import numpy as np

import concourse.bass as bass
import concourse.tile as tile
from concourse import bacc, mybir

B, N, HID, HEADS = 128, 8192, 64, 4
NCORES = 8
BPC = B // NCORES  # 16 batch rows per core
CHUNK = 512
NCH = N // CHUNK  # 16
NB = N // 128  # 64 point-blocks of 128

F32 = mybir.dt.float32
BF16 = mybir.dt.bfloat16
AF = mybir.ActivationFunctionType
ALU = mybir.AluOpType


def build_nc(reps=1, phases="ATPD"):
    from contextlib import ExitStack

    nc = bacc.Bacc()
    f32 = F32

    xp_d = nc.dram_tensor("xp", [BPC, 6, N], BF16, kind="ExternalInput")
    xpa_d = nc.dram_tensor("xpa", [NCH, 4, BPC * CHUNK], BF16, kind="ExternalInput")
    cb1_d = nc.dram_tensor("cb1", [128, BPC], f32, kind="ExternalInput")
    cb2_d = nc.dram_tensor("cb2", [128, BPC], f32, kind="ExternalInput")
    wp_d = nc.dram_tensor("wp", [4, 256], BF16, kind="ExternalInput")
    w2a_d = nc.dram_tensor("w2a", [128, BPC * 64], BF16, kind="ExternalInput")
    w2b_d = nc.dram_tensor("w2b", [128, BPC * 64], BF16, kind="ExternalInput")
    wenm_d = nc.dram_tensor("wenm", [6, BPC * 65], BF16, kind="ExternalInput")
    ew2b_d = nc.dram_tensor("ew2b", [65, 64], f32, kind="ExternalInput")
    ow1_d = nc.dram_tensor("ow1", [64, 256], f32, kind="ExternalInput")
    ob1_d = nc.dram_tensor("ob1", [1, 64], f32, kind="ExternalInput")
    w2o_d = nc.dram_tensor("w2o", [65, 1], f32, kind="ExternalInput")
    id64_d = nc.dram_tensor("id64", [64, 64], BF16, kind="ExternalInput")
    id64f_d = nc.dram_tensor("id64f", [64, 64], f32, kind="ExternalInput")
    on16_d = nc.dram_tensor("on16", [1, BPC], f32, kind="ExternalInput")
    out_d = nc.dram_tensor("out", [BPC], f32, kind="ExternalOutput")

    with tile.TileContext(nc) as tc, ExitStack() as ctx:
        consts = ctx.enter_context(tc.tile_pool(name="consts", bufs=1))

        def cload(dram, shape, nm, dt=f32):
            t = consts.tile(shape, dt, name=nm, tag=nm)
            nc.sync.dma_start(t[:], dram[:])
            return t

        wp_s = cload(wp_d, [4, 256], "wp_s", BF16)
        cb1_s = cload(cb1_d, [128, BPC], "cb1_s")
        cb2_s = cload(cb2_d, [128, BPC], "cb2_s")
        w2a_s = cload(w2a_d, [128, BPC * 64], "w2a_s", BF16)
        w2b_s = cload(w2b_d, [128, BPC * 64], "w2b_s", BF16)
        wenm_s = cload(wenm_d, [6, BPC * 65], "wenm_s", BF16)
        ew2b_s = cload(ew2b_d, [65, 64], "ew2b_s")
        ow1_s = cload(ow1_d, [64, 256], "ow1_s")
        ob1_s = cload(ob1_d, [1, 64], "ob1_s")
        w2o_s = cload(w2o_d, [65, 1], "w2o_s")
        id64_s = cload(id64_d, [64, 64], "id64_s", BF16)
        id64f_s = cload(id64f_d, [64, 64], "id64f_s")
        on16_s = cload(on16_d, [1, BPC], "on16_s")

        # n-major exp(logits): block t occupies cols [t*64, (t+1)*64), within a
        # block: partition p = n offset, col = 4*b + h
        enm = consts.tile([128, NB * 64], BF16, name="enm", tag="enm")
        ctxnT = consts.tile([65, 64], f32, name="ctxnT", tag="ctxnT")
        obuf = consts.tile([65, BPC], f32, name="obuf", tag="obuf")
        fct = consts.tile([64, 64], f32, name="fct", tag="fct")
        res = consts.tile([1, BPC], f32, name="res", tag="res")
        nc.vector.memset(ctxnT[64:65, :], 1.0)
        nc.vector.memset(obuf[64:65, :], 1.0)

        if "A" not in phases:
            nc.vector.memset(enm[:, 0:8], 0.0)
        for _rep in range(reps):
            _build_body(
                nc, tc, xp_d, xpa_d, out_d,
                wp_s, cb1_s, cb2_s, w2a_s, w2b_s, wenm_s, ew2b_s, ow1_s,
                ob1_s, w2o_s, id64_s, id64f_s, on16_s,
                enm, ctxnT, obuf, fct, res, phases,
            )

    if not nc.is_finalized():
        nc.finalize()
    return nc


def _build_body(
    nc, tc, xp_d, xpa_d, out_d,
    wp_s, cb1_s, cb2_s, w2a_s, w2b_s, wenm_s, ew2b_s, ow1_s,
    ob1_s, w2o_s, id64_s, id64f_s, on16_s,
    enm, ctxnT, obuf, fct, res, phases="ATPD",
):
    from contextlib import ExitStack

    f32 = F32
    if "A" in phases:
        # ---- Phase A: score-net hidden + logits (feature-major) ----------
        with ExitStack() as pctx:
            xpool = pctx.enter_context(tc.tile_pool(name="xpA", bufs=3))
            shpool = pctx.enter_context(tc.tile_pool(name="shp", bufs=4))
            epool = pctx.enter_context(tc.tile_pool(name="ep", bufs=2))
            psA = pctx.enter_context(tc.tile_pool(name="psA", bufs=2, space="PSUM"))
            psL = pctx.enter_context(tc.tile_pool(name="psL", bufs=2, space="PSUM"))
            psT = pctx.enter_context(tc.tile_pool(name="psT", bufs=2, space="PSUM"))

            xpcs = {}

            def load_xpc(c):
                t = xpool.tile([4, BPC * CHUNK], BF16, name="xpc", tag="xpc")
                nc.sync.dma_start(t[:], xpa_d[c])
                xpcs[c] = t

            DEPTH = 2  # software-pipeline depth: sh-MMs run DEPTH b's ahead
            lg_done = {}

            def expose(c):
                # exp of chunk c's logits, then transpose its 4 blocks n-major
                lg = lg_done.pop(c)
                e_c = epool.tile([64, CHUNK], BF16, name="e_c", tag="e_c")
                nc.scalar.activation(e_c[:], lg[:], AF.Exp)
                for j in range(CHUNK // 128):
                    t = c * (CHUNK // 128) + j
                    t_ps = psT.tile([128, 64], BF16, name="t_ps", tag="tp")
                    nc.tensor.transpose(
                        t_ps[:], e_c[:, j * 128 : (j + 1) * 128], id64_s[:]
                    )
                    nc.vector.tensor_copy(
                        out=enm[:, t * 64 : (t + 1) * 64], in_=t_ps[:]
                    )

            load_xpc(0)
            if NCH > 1:
                load_xpc(1)
            for c in range(NCH):
                cs = slice(c * CHUNK, (c + 1) * CHUNK)
                if c + 2 < NCH:
                    load_xpc(c + 2)
                if c > 0:
                    expose(c - 1)
                xpc = xpcs.pop(c)
                lg_ps = psL.tile([64, CHUNK], f32, name="lg_ps", tag="lg")
                pend = []

                def drain_lg(lg_ps=lg_ps):
                    b, s1, s2 = pend.pop(0)
                    nc.tensor.matmul(
                        lg_ps[:],
                        w2a_s[:, b * 64 : (b + 1) * 64],
                        s1[:],
                        start=(b == 0),
                        stop=False,
                        skip_group_check=True,
                    )
                    nc.tensor.matmul(
                        lg_ps[:],
                        w2b_s[:, b * 64 : (b + 1) * 64],
                        s2[:],
                        start=False,
                        stop=(b == BPC - 1),
                        skip_group_check=True,
                    )

                for b in range(BPC):
                    xb = xpc[:, b * CHUNK : (b + 1) * CHUNK]
                    sh1_ps = psA.tile([128, CHUNK], f32, name="sh1_ps", tag="sh1")
                    nc.tensor.matmul(
                        sh1_ps[:], wp_s[:, 0:128], xb[:, :], start=True, stop=True
                    )
                    sh2_ps = psA.tile([128, CHUNK], f32, name="sh2_ps", tag="sh2")
                    nc.tensor.matmul(
                        sh2_ps[:], wp_s[:, 128:256], xb[:, :], start=True, stop=True
                    )
                    sh1_sb = shpool.tile([128, CHUNK], BF16, name="sh1_sb", tag="sh1s")
                    sh2_sb = shpool.tile([128, CHUNK], BF16, name="sh2_sb", tag="sh2s")
                    if b % 2 == 0:
                        nc.scalar.activation(
                            sh1_sb[:], sh1_ps[:], AF.Relu, bias=cb1_s[:, b : b + 1]
                        )
                        nc.vector.tensor_scalar(
                            sh2_sb[:], sh2_ps[:], cb2_s[:, b : b + 1], 0.0,
                            ALU.add, ALU.max,
                        )
                    else:
                        nc.vector.tensor_scalar(
                            sh1_sb[:], sh1_ps[:], cb1_s[:, b : b + 1], 0.0,
                            ALU.add, ALU.max,
                        )
                        nc.scalar.activation(
                            sh2_sb[:], sh2_ps[:], AF.Relu, bias=cb2_s[:, b : b + 1]
                        )
                    pend.append((b, sh1_sb, sh2_sb))
                    if len(pend) > DEPTH:
                        drain_lg()
                while pend:
                    drain_lg()
                lg_done[c] = lg_ps
            expose(NCH - 1)

    if "P" in phases:
        # ---- Phase C2: n-major encoder hidden + weighted pooling ---------
        with ExitStack() as pctx:
            xbpool = pctx.enter_context(tc.tile_pool(name="xpC", bufs=2))
            h1pool = pctx.enter_context(tc.tile_pool(name="h1p", bufs=3))
            smpool = pctx.enter_context(tc.tile_pool(name="smp", bufs=2))
            psH = pctx.enter_context(tc.tile_pool(name="psH", bufs=3, space="PSUM"))
            psC = pctx.enter_context(tc.tile_pool(name="psC", bufs=2, space="PSUM"))
            psU = pctx.enter_context(tc.tile_pool(name="psU", bufs=2, space="PSUM"))
            TB = 4  # blocks per psum batch
            xpbs = {}

            def load_xpb(b):
                t = xbpool.tile([6, N], BF16, name="xpb", tag="xpb")
                nc.sync.dma_start(t[:], xp_d[b])
                xpbs[b] = t

            load_xpb(0)
            for b in range(BPC):
                if b + 1 < BPC:
                    load_xpb(b + 1)
                xpb = xpbs.pop(b)
                c1_ps = psC.tile([4, 65], f32, name="c1_ps", tag="c1")
                hpend = []

                def drain_pool(c1_ps=c1_ps, b=b):
                    tg, h1_sb = hpend.pop(0)
                    for j in range(TB):
                        t = tg * TB + j
                        nc.tensor.matmul(
                            c1_ps[:],
                            enm[:, t * 64 + 4 * b : t * 64 + 4 * b + 4],
                            h1_sb[:, j * 65 : (j + 1) * 65],
                            start=(t == 0),
                            stop=(t == NB - 1),
                            skip_group_check=True,
                        )

                for tg in range(NB // TB):
                    h1_ps = psH.tile([128, TB * 65], f32, name="h1_ps", tag="h1")
                    for j in range(TB):
                        t = tg * TB + j
                        nc.tensor.matmul(
                            h1_ps[:, j * 65 : (j + 1) * 65],
                            xpb[:, t * 128 : (t + 1) * 128],
                            wenm_s[:, b * 65 : (b + 1) * 65],
                            start=True,
                            stop=True,
                            skip_group_check=True,
                        )
                    h1_sb = h1pool.tile([128, TB * 65], BF16, name="h1_sb", tag="h1s")
                    nc.vector.tensor_scalar(
                        h1_sb[:], h1_ps[:], 0.0, None, ALU.max
                    )
                    hpend.append((tg, h1_sb))
                    if len(hpend) > 1:
                        drain_pool()
                while hpend:
                    drain_pool()
                rz = smpool.tile([4, 1], f32, name="rz", tag="rz")
                nc.vector.reciprocal(rz[:], c1_ps[:, 64:65])
                ctxn = smpool.tile([4, 64], f32, name="ctxn", tag="ctxn")
                nc.vector.tensor_scalar_mul(ctxn[:], c1_ps[:, 0:64], rz[:])
                tp_ps = psU.tile([64, 4], f32, name="tp_ps", tag="tp2")
                nc.tensor.transpose(tp_ps[:], ctxn[:], id64f_s[0:4, 0:4])
                nc.vector.tensor_copy(
                    out=ctxnT[0:64, b * 4 : (b + 1) * 4], in_=tp_ps[:]
                )

    if "D" in phases:
        # ---- Phase D: pooled-context encoder layer 2 + output MLP --------
        with ExitStack() as pctx:
            psD = pctx.enter_context(tc.tile_pool(name="psD", bufs=1, space="PSUM"))
            fct_ps = psD.tile([64, 64], f32, name="fct_ps", tag="fctp")
            nc.tensor.matmul(fct_ps[:], ew2b_s[:], ctxnT[:], start=True, stop=True)
            nc.vector.tensor_copy(out=fct[:], in_=fct_ps[:])
            fct_bh = fct.rearrange("d (b h) -> d b h", h=HEADS)
            o1_ps = psD.tile([64, BPC], f32, name="o1_ps", tag="o1p")
            for h in range(HEADS):
                nc.tensor.matmul(
                    o1_ps[:],
                    ow1_s[:, h * 64 : (h + 1) * 64],
                    fct_bh[:, :, h],
                    start=(h == 0),
                    stop=False,
                    skip_group_check=True,
                )
            nc.tensor.matmul(
                o1_ps[:], ob1_s[:], on16_s[:], start=False, stop=True,
                skip_group_check=True,
            )
            nc.scalar.activation(obuf[0:64, :], o1_ps[:], AF.Relu)
            fin_ps = psD.tile([1, BPC], f32, name="fin_ps", tag="finp")
            nc.tensor.matmul(fin_ps[:], w2o_s[:], obuf[:], start=True, stop=True)
            nc.vector.tensor_copy(out=res[:], in_=fin_ps[:])
            nc.sync.dma_start(out_d.rearrange("(a n) -> a n", a=1), res[:])


def make_in_maps(inputs):
    """Host-side marshalling: slice B across cores and pack weights into the
    layouts the device program expects.

    bf16 note: the big streamed matmuls run in bf16.  To avoid systematic
    model-weight rounding, layer-1 weights are split hi/lo across extra
    contraction rows (w = hi + lo with both bf16); per-point input rounding
    is stochastic and averages out in the softmax pooling."""
    import ml_dtypes

    bf = ml_dtypes.bfloat16
    f = np.float32

    def split(a):
        hi = a.astype(bf)
        lo = (a - hi.astype(f)).astype(bf)
        return hi, lo
    radar = np.concatenate(
        [np.asarray(inputs["radar_xy"], f), np.asarray(inputs["radar_dir"], f)], axis=1
    )  # [B, 4]
    pts = np.asarray(inputs["pts"], f)
    enc_w1 = np.asarray(inputs["enc_w1"], f)
    enc_b1 = np.asarray(inputs["enc_b1"], f)
    enc_w2 = np.asarray(inputs["enc_w2"], f)
    enc_b2 = np.asarray(inputs["enc_b2"], f)
    sc_w1 = np.asarray(inputs["sc_w1"], f)
    sc_b1 = np.asarray(inputs["sc_b1"], f)
    sc_w2 = np.asarray(inputs["sc_w2"], f)
    out_w1 = np.asarray(inputs["out_w1"], f)
    out_b1 = np.asarray(inputs["out_b1"], f)
    out_w2 = np.asarray(inputs["out_w2"], f)
    out_b2 = np.asarray(inputs["out_b2"], f)

    # per-b layer-1 bias vectors (radar is constant over n)
    cb_sc = np.einsum("br,hrd->bhd", radar, sc_w1[:, :4, :]) + sc_b1  # [B, 4, 64]
    cb_enc = radar @ enc_w1[:4] + enc_b1  # [B, 64]

    # xp rows: [xh, yh, xh, yh, 1, 1] (bf16); rows 0-3 feed the weight-split
    # layer-1 matmuls, rows 4-5 carry the (split) bias contraction.
    xp = np.empty((B, 6, N), bf)
    xh = pts[:, :, 0].astype(bf)
    yh = pts[:, :, 1].astype(bf)
    xp[:, 0] = xh
    xp[:, 1] = yh
    xp[:, 2] = xh
    xp[:, 3] = yh
    xp[:, 4] = 1.0
    xp[:, 5] = 1.0

    # wp rows: [wxh, wyh, wxl, wyl] against xp rows [xh, yh, xh, yh]
    wp = np.empty((4, 256), bf)
    for h in range(HEADS):
        wxh, wxl = split(sc_w1[h, 4, :])
        wyh, wyl = split(sc_w1[h, 5, :])
        wp[0, h * 64 : (h + 1) * 64] = wxh
        wp[1, h * 64 : (h + 1) * 64] = wyh
        wp[2, h * 64 : (h + 1) * 64] = wxl
        wp[3, h * 64 : (h + 1) * 64] = wyl
    # heads 0,1 feed sh1 (wp cols 0:128), heads 2,3 feed sh2 (cols 128:256)

    w2a = np.zeros((128, BPC * 64), bf)
    w2b = np.zeros((128, BPC * 64), bf)
    for bl in range(BPC):
        w2a[0:64, bl * 64 + 4 * bl + 0] = sc_w2[0]
        w2a[64:128, bl * 64 + 4 * bl + 1] = sc_w2[1]
        w2b[0:64, bl * 64 + 4 * bl + 2] = sc_w2[2]
        w2b[64:128, bl * 64 + 4 * bl + 3] = sc_w2[3]

    ew2b = np.concatenate([enc_w2, enc_b2[None, :]], axis=0)  # [65, 64]
    ow1 = np.empty((64, 256), f)
    for h in range(HEADS):
        ow1[:, h * 64 : (h + 1) * 64] = out_w1[h * 64 : (h + 1) * 64, :]
    ob1 = np.ascontiguousarray(out_b1[None, :])
    w2o = np.concatenate([out_w2, out_b2[None, :]], axis=0)  # [65, 1]
    id64 = np.eye(64, dtype=bf)
    id64f = np.eye(64, dtype=f)
    on16 = np.ones((1, BPC), f)

    in_maps = []
    for c in range(NCORES):
        sl = slice(c * BPC, (c + 1) * BPC)
        cb1 = np.ascontiguousarray(cb_sc[sl, 0:2].reshape(BPC, 128).T)
        cb2 = np.ascontiguousarray(cb_sc[sl, 2:4].reshape(BPC, 128).T)
        # wenm rows [wxh, wyh, wxl, wyl, bh, bl] vs xp rows [xh, yh, xh, yh, 1, 1]
        wenm = np.zeros((6, BPC * 65), bf)
        exh, exl = split(enc_w1[4])
        eyh, eyl = split(enc_w1[5])
        for bl in range(BPC):
            s = slice(bl * 65, bl * 65 + 64)
            wenm[0, s] = exh
            wenm[1, s] = eyh
            wenm[2, s] = exl
            wenm[3, s] = eyl
            bh, blo = split(cb_enc[c * BPC + bl])
            wenm[4, s] = bh
            wenm[5, s] = blo
            wenm[4, bl * 65 + 64] = 1.0
        xpc_core = np.ascontiguousarray(xp[sl])
        xpa = np.ascontiguousarray(
            xpc_core[:, 0:4]
            .reshape(BPC, 4, NCH, CHUNK)
            .transpose(2, 1, 0, 3)
            .reshape(NCH, 4, BPC * CHUNK)
        )
        in_maps.append(
            dict(
                xp=xpc_core,
                xpa=xpa,
                cb1=cb1,
                cb2=cb2,
                wp=wp,
                w2a=w2a,
                w2b=w2b,
                wenm=wenm,
                ew2b=ew2b,
                ow1=ow1,
                ob1=ob1,
                w2o=w2o,
                id64=id64,
                id64f=id64f,
                on16=on16,
            )
        )
    return in_maps


_CACHE = {}


def _get_runner():
    """Build the Bass program once and a cached jitted PJRT executable over
    the 8 cores (shard_map along axis 0 of every input)."""
    if "runner" in _CACHE:
        return _CACHE["runner"]

    import jax
    from jax.sharding import Mesh, NamedSharding, PartitionSpec

    from concourse.bass2jax import (
        _bass_exec_p,
        install_neuronx_cc_hook,
        partition_id_tensor,
        shard_map,
    )

    nc = build_nc()
    _CACHE["nc"] = nc
    install_neuronx_cc_hook()
    partition_name = nc.partition_id_tensor.name if nc.partition_id_tensor else None
    in_names, out_names, out_avals = [], [], []
    for alloc in nc.m.functions[0].allocations:
        if not isinstance(alloc, mybir.MemoryLocationSet):
            continue
        name = alloc.memorylocations[0].name
        if alloc.kind == "ExternalInput":
            if name != partition_name:
                in_names.append(name)
        elif alloc.kind == "ExternalOutput":
            out_names.append(name)
            out_avals.append(
                jax.core.ShapedArray(tuple(alloc.tensor_shape), mybir.dt.np(alloc.dtype))
            )
    all_in_names = tuple(in_names + out_names)
    if partition_name is not None:
        all_in_names = all_in_names + (partition_name,)

    def _body(*args):
        operands = list(args)
        if partition_name is not None:
            operands.append(partition_id_tensor())
        return tuple(
            _bass_exec_p.bind(
                *operands,
                out_avals=tuple(out_avals),
                in_names=all_in_names,
                out_names=tuple(out_names),
                lowering_input_output_aliases=(),
                sim_require_finite=True,
                sim_require_nnan=True,
                nc=nc,
            )
        )

    devices = jax.devices()[:NCORES]
    mesh = Mesh(np.asarray(devices), ("core",))
    nin = len(in_names) + len(out_names)
    fn = jax.jit(
        shard_map(
            _body,
            mesh=mesh,
            in_specs=(PartitionSpec("core"),) * nin,
            out_specs=(PartitionSpec("core"),) * len(out_names),
            check_rep=False,
        ),
        keep_unused=True,
    )
    sharding = NamedSharding(mesh, PartitionSpec("core"))
    runner = (fn, sharding, in_names, out_avals)
    _CACHE["runner"] = runner
    return runner


def kernel(**inputs):
    import jax

    in_maps = make_in_maps(inputs)
    fn, sharding, in_names, out_avals = _get_runner()
    concat_in = [
        np.concatenate([np.asarray(in_maps[c][name]) for c in range(NCORES)], axis=0)
        for name in in_names
    ]
    concat_zeros = [
        np.zeros((NCORES * a.shape[0], *a.shape[1:]), a.dtype) for a in out_avals
    ]
    args = [jax.device_put(a, sharding) for a in (*concat_in, *concat_zeros)]
    (out,) = fn(*args)
    return np.asarray(out).reshape(B).astype(np.float32)



# revision 4
# speedup vs baseline: 1.0932x; 1.0932x over previous
"""Trainium2 Bass kernel for nn_MultiHeadMLPAttentionModel — K=128-dense design.

All matmuls use full-K=128 operands (zero-padded stationaries / zero-padded
weight rhs) so the PE's HAM activity monitor sees dense streams and holds the
clock at 2.4 GHz for the whole kernel (K=8 row-tiled matmuls read as "idle"
and get throttled to 1.2 GHz — measured).

Per core: 16 batch rows as 8 b-pairs. Point data is replicated across all
128 SBUF partitions (8 copies of each b's 8 feature rows); stationaries are
zero except the 8 rows matching their b's home slot, so K=128 contraction
reproduces the K=8 result at identical stream cost.

Per pair (16 chunks of 512 points):
  * 4 serial K=128 fp8 matmuls -> score hiddens for both b's (bias folded
    into contraction rows; x and layer-1 weights split hi/lo for precision),
    into two [128,1024] 2-bank PSUM tiles; one [128,1024] relu op each
    (vector / scalar) -> fp8 SBUF.
  * logits point-major: 16 tiny N=4 matmuls per chunk, lhsT = sh slices,
    rhs = zero-padded w2 columns; all 64 blocks of each b accumulate into
    one [128,512] PSUM tile (cols 4t+h) -> exp once per pair -> bf16 enm.
  * encoder h1 point-major: lhsT = the (replicated) data block, two rhs
    (be / bo wenm columns) share each stationary; relu -> bf16 strips.
  * pooling of the PREVIOUS pair interleaved (8 matmuls per chunk) so the
    PE never idles between pairs.
Phase D (once): enc layer 2 + output MLP on pooled contexts -> [16].
"""

import numpy as np

import concourse.bass as bass
import concourse.tile as tile
from concourse import bacc, mybir

B, N, HID, HEADS = 128, 8192, 64, 4
NCORES = 8
BPC = B // NCORES      # 16 batch rows per core
PAIRS = BPC // 2       # 8 b-pairs
NCHB = N // 512        # 16 chunks per pair
NB = N // 128          # 64 point-blocks per b
NSTR = 16              # h1 strips per b (4 blocks each)

F32 = mybir.dt.float32
BF16 = mybir.dt.bfloat16
FP8 = mybir.dt.float8e4
AF = mybir.ActivationFunctionType
ALU = mybir.AluOpType


def build_nc():
    from contextlib import ExitStack

    nc = bacc.Bacc()
    f32 = F32

    xpa_d = nc.dram_tensor("xpa", [PAIRS, 128, N], FP8, kind="ExternalInput")
    wsc_d = nc.dram_tensor("wsc", [128, PAIRS * 512], FP8, kind="ExternalInput")
    wenm_d = nc.dram_tensor("wenm", [128, PAIRS * 130], FP8, kind="ExternalInput")
    w24_d = nc.dram_tensor("w24", [128, 8], FP8, kind="ExternalInput")
    ew2b_d = nc.dram_tensor("ew2b", [65, 64], f32, kind="ExternalInput")
    ow1_d = nc.dram_tensor("ow1", [64, 256], f32, kind="ExternalInput")
    ob1_d = nc.dram_tensor("ob1", [1, 64], f32, kind="ExternalInput")
    w2o_d = nc.dram_tensor("w2o", [65, 1], f32, kind="ExternalInput")
    id4_d = nc.dram_tensor("id4", [4, 4], f32, kind="ExternalInput")
    on16_d = nc.dram_tensor("on16", [1, BPC], f32, kind="ExternalInput")
    out_d = nc.dram_tensor("out", [BPC], f32, kind="ExternalOutput")

    with tile.TileContext(nc) as tc, ExitStack() as ctx:
        consts = ctx.enter_context(tc.tile_pool(name="consts", bufs=1))

        def cload(dram, shape, nm, dt=f32):
            t = consts.tile(shape, dt, name=nm, tag=nm)
            nc.sync.dma_start(t[:], dram[:])
            return t

        wsc = cload(wsc_d, [128, PAIRS * 512], "wsc", FP8)
        wenm = cload(wenm_d, [128, PAIRS * 130], "wenm", FP8)
        w24 = cload(w24_d, [128, 8], "w24", FP8)
        ew2b = cload(ew2b_d, [65, 64], "ew2b")
        ow1 = cload(ow1_d, [64, 256], "ow1")
        ob1 = cload(ob1_d, [1, 64], "ob1")
        w2o = cload(w2o_d, [65, 1], "w2o")
        id4 = cload(id4_d, [4, 4], "id4")
        on16 = cload(on16_d, [1, BPC], "on16")

        ctxnT = consts.tile([65, 64], f32, name="ctxnT", tag="ctxnT")
        obuf = consts.tile([65, BPC], f32, name="obuf", tag="obuf")
        fct = consts.tile([64, 64], f32, name="fct", tag="fct")
        res = consts.tile([1, BPC], f32, name="res", tag="res")
        nc.vector.memset(ctxnT[64:65, :], 1.0)
        nc.vector.memset(obuf[64:65, :], 1.0)

        with ExitStack() as pctx:
            xapool = pctx.enter_context(tc.tile_pool(name="xap", bufs=2))
            shpool = pctx.enter_context(tc.tile_pool(name="shp", bufs=3))
            h1pool = pctx.enter_context(tc.tile_pool(name="h1p", bufs=2))
            enmpool = pctx.enter_context(tc.tile_pool(name="enmp", bufs=2))
            smpool = pctx.enter_context(tc.tile_pool(name="smp", bufs=2))
            psA = pctx.enter_context(tc.tile_pool(name="psA", bufs=1, space="PSUM"))
            psB = pctx.enter_context(tc.tile_pool(name="psB", bufs=1, space="PSUM"))
            psLG = pctx.enter_context(tc.tile_pool(name="psLG", bufs=1, space="PSUM"))
            psH1 = pctx.enter_context(tc.tile_pool(name="psH1", bufs=1, space="PSUM"))
            psCX = pctx.enter_context(tc.tile_pool(name="psCX", bufs=1, space="PSUM"))

            xas = {}

            def load_xa(p):
                t = xapool.tile([128, N], FP8, name="xa", tag="xa")
                nc.sync.dma_start(t[:], xpa_d[p])
                xas[p] = t

            load_xa(0)
            if PAIRS > 1:
                load_xa(1)

            def pool_mms(cx, enm, h1a, ts):
                for t in ts:
                    for i in (0, 1):
                        off = 260 * (NSTR * i + t // 4) + 65 * (t % 4)
                        nc.tensor.matmul(
                            cx[0:4, 65 * i : 65 * i + 65],
                            enm[:, 256 * i + 4 * t : 256 * i + 4 * t + 4],
                            h1a[:, off : off + 65],
                            start=(t == 0), stop=(t == NB - 1),
                            skip_group_check=True,
                        )

            def epilogue(cx, p):
                rz = smpool.tile([4, 2], f32, name="rz", tag="rz")
                nc.vector.reciprocal(rz[:, 0:1], cx[0:4, 64:65])
                nc.vector.reciprocal(rz[:, 1:2], cx[0:4, 129:130])
                cpn = smpool.tile([4, 128], f32, name="cpn", tag="cpn")
                nc.vector.tensor_scalar_mul(cpn[:, 0:64], cx[0:4, 0:64], rz[:, 0:1])
                nc.vector.tensor_scalar_mul(cpn[:, 64:128], cx[0:4, 65:129], rz[:, 1:2])
                tp = cx[:, 130:134]
                nc.tensor.transpose(tp[:], cpn[:], id4[:])
                nc.vector.tensor_copy(
                    out=ctxnT[0:64, 8 * p : 8 * p + 4], in_=tp[0:64, :]
                )
                nc.vector.tensor_copy(
                    out=ctxnT[0:64, 8 * p + 4 : 8 * p + 8], in_=tp[64:128, :]
                )

            prev = None
            for p in range(PAIRS):
                if p + 2 < PAIRS:
                    load_xa(p + 2)
                xa = xas.pop(p)
                lg = psLG.tile([128, 512], f32, name="lg", tag="lg")
                h1a = h1pool.tile([128, 2 * NSTR * 260], BF16, name="h1a", tag="h1a")
                if prev is not None:
                    cxp = psCX.tile([128, 134], f32, name="cx", tag="cx")

                for cc in range(NCHB):
                    cs = slice(512 * cc, 512 * (cc + 1))
                    # ---- score hiddens: 4 serial K=128 fp8 matmuls.
                    # Alternate which psum tile is written first each chunk so
                    # each tile gets ~1.5 periods between generations (pseudo
                    # double-buffering within the 4-bank budget).
                    sAB = psA.tile([128, 1024], f32, name="sAB", tag="sAB")
                    sCD = psB.tile([128, 1024], f32, name="sCD", tag="sCD")
                    order = (0, 1, 2, 3) if cc % 2 == 0 else (2, 3, 0, 1)
                    for s in order:
                        dst = (sAB if s < 2 else sCD)[:, 512 * (s % 2) : 512 * (s % 2) + 512]
                        nc.tensor.matmul(
                            dst,
                            wsc[:, 512 * p + 128 * s : 512 * p + 128 * s + 128],
                            xa[:, cs],
                            start=True, stop=True,
                            skip_group_check=True,
                        )
                    sh_sb = shpool.tile([128, 2048], FP8, name="sh_sb", tag="shs")
                    if cc % 2 == 0:
                        nc.vector.tensor_scalar(sh_sb[:, 0:1024], sAB[:], 0.0, None, ALU.max)
                        nc.scalar.activation(sh_sb[:, 1024:2048], sCD[:], AF.Relu)
                    else:
                        nc.scalar.activation(sh_sb[:, 1024:2048], sCD[:], AF.Relu)
                        nc.vector.tensor_scalar(sh_sb[:, 0:1024], sAB[:], 0.0, None, ALU.max)

                    # ---- pooling of the PREVIOUS pair (keeps PE dense)
                    if prev is not None:
                        pool_mms(cxp, prev[0], prev[1], (4 * cc, 4 * cc + 1, 4 * cc + 2, 4 * cc + 3))

                    # ---- logits
                    for i in (0, 1):
                        for j in range(4):
                            t = 4 * cc + j
                            lgo = lg[:, 256 * i + 4 * t : 256 * i + 4 * t + 4]
                            base = 1024 * i
                            nc.tensor.matmul(
                                lgo,
                                sh_sb[:, base + 128 * j : base + 128 * j + 128],
                                w24[:, 0:4],
                                start=True, stop=False,
                                skip_group_check=True,
                            )
                            nc.tensor.matmul(
                                lgo,
                                sh_sb[:, base + 512 + 128 * j : base + 512 + 128 * j + 128],
                                w24[:, 4:8],
                                start=False, stop=True,
                                skip_group_check=True,
                            )

                    # ---- encoder h1: shared data stationary, two zero-padded
                    # weight rhs (be rows 0-7 / bo rows 64-71)
                    h1A = psH1.tile([128, 260], f32, name="h1A", tag="h1A")
                    h1B = psH1.tile([128, 260], f32, name="h1B", tag="h1B")
                    for j in range(4):
                        t = 4 * cc + j
                        for ht, o in ((h1A, 0), (h1B, 65)):
                            nc.tensor.matmul(
                                ht[:, 65 * j : 65 * j + 65],
                                xa[:, 128 * t : 128 * t + 128],
                                wenm[:, 130 * p + o : 130 * p + o + 65],
                                start=True, stop=True,
                                skip_group_check=True,
                            )
                    hsA = h1a[:, 260 * cc : 260 * (cc + 1)]
                    hsB = h1a[:, 260 * (NSTR + cc) : 260 * (NSTR + cc + 1)]
                    if cc % 2 == 0:
                        nc.vector.tensor_scalar(hsA, h1A[:], 0.0, None, ALU.max)
                        nc.scalar.activation(hsB, h1B[:], AF.Relu)
                    else:
                        nc.scalar.activation(hsA, h1A[:], AF.Relu)
                        nc.vector.tensor_scalar(hsB, h1B[:], 0.0, None, ALU.max)

                if prev is not None:
                    epilogue(cxp, p - 1)
                enm = enmpool.tile([128, 512], BF16, name="enm", tag="enm")
                nc.scalar.activation(enm[:], lg[:], AF.Exp)
                prev = (enm, h1a)

            cxp = psCX.tile([128, 134], f32, name="cx", tag="cx")
            for cc in range(NCHB):
                pool_mms(cxp, prev[0], prev[1], (4 * cc, 4 * cc + 1, 4 * cc + 2, 4 * cc + 3))
            epilogue(cxp, PAIRS - 1)

        # ---- Phase D: pooled-context encoder layer 2 + output MLP
        with ExitStack() as pctx:
            psD = pctx.enter_context(tc.tile_pool(name="psD", bufs=1, space="PSUM"))
            fct_ps = psD.tile([64, 64], f32, name="fct_ps", tag="fctp")
            nc.tensor.matmul(fct_ps[:], ew2b[:], ctxnT[:], start=True, stop=True)
            nc.vector.tensor_copy(out=fct[:], in_=fct_ps[:])
            fct_bh = fct.rearrange("d (b h) -> d b h", h=HEADS)
            o1_ps = psD.tile([64, BPC], f32, name="o1_ps", tag="o1p")
            for h in range(HEADS):
                nc.tensor.matmul(
                    o1_ps[:],
                    ow1[:, h * 64 : (h + 1) * 64],
                    fct_bh[:, :, h],
                    start=(h == 0),
                    stop=False,
                    skip_group_check=True,
                )
            nc.tensor.matmul(
                o1_ps[:], ob1[:], on16[:], start=False, stop=True,
                skip_group_check=True,
            )
            nc.scalar.activation(obuf[0:64, :], o1_ps[:], AF.Relu)
            fin_ps = psD.tile([1, BPC], f32, name="fin_ps", tag="finp")
            nc.tensor.matmul(fin_ps[:], w2o[:], obuf[:], start=True, stop=True)
            nc.vector.tensor_copy(out=res[:], in_=fin_ps[:])
            nc.sync.dma_start(out_d.rearrange("(a n) -> a n", a=1), res[:])

    if not nc.is_finalized():
        nc.finalize()
    return nc


def make_in_maps(inputs):
    """Host-side marshalling: fp8 hi/lo packing; radar folded into biases;
    point rows replicated across all 128 partitions (8 copies per b)."""
    import ml_dtypes

    f8 = ml_dtypes.float8_e4m3fn
    f = np.float32

    def split8(a):
        hi = a.astype(f8)
        lo = (a - hi.astype(f)).astype(f8)
        return hi, lo

    radar = np.concatenate(
        [np.asarray(inputs["radar_xy"], f), np.asarray(inputs["radar_dir"], f)], axis=1
    )
    pts = np.asarray(inputs["pts"], f)
    enc_w1 = np.asarray(inputs["enc_w1"], f)
    enc_b1 = np.asarray(inputs["enc_b1"], f)
    enc_w2 = np.asarray(inputs["enc_w2"], f)
    enc_b2 = np.asarray(inputs["enc_b2"], f)
    sc_w1 = np.asarray(inputs["sc_w1"], f)
    sc_b1 = np.asarray(inputs["sc_b1"], f)
    sc_w2 = np.asarray(inputs["sc_w2"], f)
    out_w1 = np.asarray(inputs["out_w1"], f)
    out_b1 = np.asarray(inputs["out_b1"], f)
    out_w2 = np.asarray(inputs["out_w2"], f)
    out_b2 = np.asarray(inputs["out_b2"], f)

    cb_sc = np.einsum("br,hrd->bhd", radar, sc_w1[:, :4, :]) + sc_b1  # [B, 4, 64]
    cb_enc = radar @ enc_w1[:4] + enc_b1  # [B, 64]

    x = pts[:, :, 0]
    y = pts[:, :, 1]
    xh, xl = split8(x)
    yh, yl = split8(y)

    # 8 feature rows per b: [xh, yh, xh, yh, xl, yl, 1, 1]
    xrows = np.empty((B, 8, N), f8)
    xrows[:, 0] = xh
    xrows[:, 1] = yh
    xrows[:, 2] = xh
    xrows[:, 3] = yh
    xrows[:, 4] = xl
    xrows[:, 5] = yl
    xrows[:, 6] = 1.0
    xrows[:, 7] = 1.0

    # score stationary rows: [wxh, wyh, wxl, wyl, wxh, wyh, cbh, cbl]
    wx = sc_w1[:, 4, :]
    wy = sc_w1[:, 5, :]
    wxh_, wxl_ = split8(wx)
    wyh_, wyl_ = split8(wy)

    def sc_stat(b, half, row0):
        st = np.zeros((128, 128), f8)
        for hh in range(2):
            h = half * 2 + hh
            s = slice(hh * 64, hh * 64 + 64)
            st[row0 + 0, s] = wxh_[h]
            st[row0 + 1, s] = wyh_[h]
            st[row0 + 2, s] = wxl_[h]
            st[row0 + 3, s] = wyl_[h]
            st[row0 + 4, s] = wxh_[h]
            st[row0 + 5, s] = wyh_[h]
            cbh, cbl = split8(cb_sc[b, h])
            st[row0 + 6, s] = cbh
            st[row0 + 7, s] = cbl
        return st

    exh_, exl_ = split8(enc_w1[4])
    eyh_, eyl_ = split8(enc_w1[5])

    def enc_rhs(b, row0):
        st = np.zeros((128, 65), f8)
        st[row0 + 0, :64] = exh_
        st[row0 + 1, :64] = eyh_
        st[row0 + 2, :64] = exl_
        st[row0 + 3, :64] = eyl_
        st[row0 + 4, :64] = exh_
        st[row0 + 5, :64] = eyh_
        cbh, cbl = split8(cb_enc[b])
        st[row0 + 6, :64] = cbh
        st[row0 + 7, :64] = cbl
        st[row0 + 6, 64] = 1.0
        return st

    w24 = np.zeros((128, 8), f8)
    w24[0:64, 0] = sc_w2[0].astype(f8)
    w24[64:128, 1] = sc_w2[1].astype(f8)
    w24[0:64, 6] = sc_w2[2].astype(f8)
    w24[64:128, 7] = sc_w2[3].astype(f8)

    ew2b = np.concatenate([enc_w2, enc_b2[None, :]], axis=0)
    ow1 = np.empty((64, 256), f)
    for h in range(HEADS):
        ow1[:, h * 64 : (h + 1) * 64] = out_w1[h * 64 : (h + 1) * 64, :]
    ob1 = np.ascontiguousarray(out_b1[None, :])
    w2o = np.concatenate([out_w2, out_b2[None, :]], axis=0)
    id4 = np.eye(4, dtype=f)
    on16 = np.ones((1, BPC), f)

    in_maps = []
    for core in range(NCORES):
        b0 = core * BPC
        xpa = np.empty((PAIRS, 128, N), f8)
        wsc = np.empty((128, PAIRS * 512), f8)
        wenm = np.empty((128, PAIRS * 130), f8)
        for p in range(PAIRS):
            be, bo = b0 + 2 * p, b0 + 2 * p + 1
            xpa[p, 0:64] = np.tile(xrows[be], (8, 1))
            xpa[p, 64:128] = np.tile(xrows[bo], (8, 1))
            wsc[:, 512 * p + 0 : 512 * p + 128] = sc_stat(be, 0, 0)
            wsc[:, 512 * p + 128 : 512 * p + 256] = sc_stat(be, 1, 0)
            wsc[:, 512 * p + 256 : 512 * p + 384] = sc_stat(bo, 0, 64)
            wsc[:, 512 * p + 384 : 512 * p + 512] = sc_stat(bo, 1, 64)
            wenm[:, 130 * p : 130 * p + 65] = enc_rhs(be, 0)
            wenm[:, 130 * p + 65 : 130 * p + 130] = enc_rhs(bo, 64)
        in_maps.append(
            dict(
                xpa=xpa, wsc=wsc, wenm=wenm, w24=w24, ew2b=ew2b, ow1=ow1,
                ob1=ob1, w2o=w2o, id4=id4, on16=on16,
            )
        )
    return in_maps


_CACHE = {}


def _get_runner():
    if "runner" in _CACHE:
        return _CACHE["runner"]

    import jax
    from jax.sharding import Mesh, NamedSharding, PartitionSpec

    from concourse.bass2jax import (
        _bass_exec_p,
        install_neuronx_cc_hook,
        partition_id_tensor,
        shard_map,
    )

    nc = build_nc()
    _CACHE["nc"] = nc
    install_neuronx_cc_hook()
    partition_name = nc.partition_id_tensor.name if nc.partition_id_tensor else None
    in_names, out_names, out_avals = [], [], []
    for alloc in nc.m.functions[0].allocations:
        if not isinstance(alloc, mybir.MemoryLocationSet):
            continue
        name = alloc.memorylocations[0].name
        if alloc.kind == "ExternalInput":
            if name != partition_name:
                in_names.append(name)
        elif alloc.kind == "ExternalOutput":
            out_names.append(name)
            out_avals.append(
                jax.core.ShapedArray(tuple(alloc.tensor_shape), mybir.dt.np(alloc.dtype))
            )
    all_in_names = tuple(in_names + out_names)
    if partition_name is not None:
        all_in_names = all_in_names + (partition_name,)

    def _body(*args):
        operands = list(args)
        if partition_name is not None:
            operands.append(partition_id_tensor())
        return tuple(
            _bass_exec_p.bind(
                *operands,
                out_avals=tuple(out_avals),
                in_names=all_in_names,
                out_names=tuple(out_names),
                lowering_input_output_aliases=(),
                sim_require_finite=True,
                sim_require_nnan=True,
                nc=nc,
            )
        )

    devices = jax.devices()[:NCORES]
    mesh = Mesh(np.asarray(devices), ("core",))
    nin = len(in_names) + len(out_names)
    fn = jax.jit(
        shard_map(
            _body,
            mesh=mesh,
            in_specs=(PartitionSpec("core"),) * nin,
            out_specs=(PartitionSpec("core"),) * len(out_names),
            check_rep=False,
        ),
        keep_unused=True,
    )
    sharding = NamedSharding(mesh, PartitionSpec("core"))
    runner = (fn, sharding, in_names, out_avals)
    _CACHE["runner"] = runner
    return runner


def kernel(**inputs):
    import jax

    in_maps = make_in_maps(inputs)
    fn, sharding, in_names, out_avals = _get_runner()
    concat_in = [
        np.concatenate([np.asarray(in_maps[c][name]) for c in range(NCORES)], axis=0)
        for name in in_names
    ]
    concat_zeros = [
        np.zeros((NCORES * a.shape[0], *a.shape[1:]), a.dtype) for a in out_avals
    ]
    args = [jax.device_put(a, sharding) for a in (*concat_in, *concat_zeros)]
    (out,) = fn(*args)
    return np.asarray(out).reshape(B).astype(np.float32)


# revision 5
# speedup vs baseline: 1.0950x; 1.0016x over previous
"""Trainium2 Bass kernel for nn_MultiHeadMLPAttentionModel — K=128-dense design.

All matmuls use full-K=128 operands (zero-padded stationaries / zero-padded
weight rhs) so the PE's HAM activity monitor sees dense streams and holds the
clock at 2.4 GHz for the whole kernel (K=8 row-tiled matmuls read as "idle"
and get throttled to 1.2 GHz — measured).

Per core: 16 batch rows as 8 b-pairs. Point data is replicated across all
128 SBUF partitions (8 copies of each b's 8 feature rows); stationaries are
zero except the 8 rows matching their b's home slot, so K=128 contraction
reproduces the K=8 result at identical stream cost.

Per pair (16 chunks of 512 points):
  * 4 serial K=128 fp8 matmuls -> score hiddens for both b's (bias folded
    into contraction rows; x and layer-1 weights split hi/lo for precision),
    into two [128,1024] 2-bank PSUM tiles; one [128,1024] relu op each
    (vector / scalar) -> fp8 SBUF.
  * logits point-major: 16 tiny N=4 matmuls per chunk, lhsT = sh slices,
    rhs = zero-padded w2 columns; all 64 blocks of each b accumulate into
    one [128,512] PSUM tile (cols 4t+h) -> exp once per pair -> bf16 enm.
  * encoder h1 point-major: lhsT = the (replicated) data block, two rhs
    (be / bo wenm columns) share each stationary; relu -> bf16 strips.
  * pooling of the PREVIOUS pair interleaved (8 matmuls per chunk) so the
    PE never idles between pairs.
Phase D (once): enc layer 2 + output MLP on pooled contexts -> [16].
"""

import numpy as np

import concourse.bass as bass
import concourse.tile as tile
from concourse import bacc, mybir

B, N, HID, HEADS = 128, 8192, 64, 4
NCORES = 8
BPC = B // NCORES      # 16 batch rows per core
PAIRS = BPC // 2       # 8 b-pairs
NCHB = N // 512        # 16 chunks per pair
NB = N // 128          # 64 point-blocks per b
NSTR = 16              # h1 strips per b (4 blocks each)

F32 = mybir.dt.float32
BF16 = mybir.dt.bfloat16
FP8 = mybir.dt.float8e4
AF = mybir.ActivationFunctionType
ALU = mybir.AluOpType


def build_nc():
    from contextlib import ExitStack

    nc = bacc.Bacc()
    f32 = F32

    xpa_d = nc.dram_tensor("xpa", [PAIRS, 128, N], FP8, kind="ExternalInput")
    wsc_d = nc.dram_tensor("wsc", [128, PAIRS * 512], FP8, kind="ExternalInput")
    wenm_d = nc.dram_tensor("wenm", [128, PAIRS * 130], FP8, kind="ExternalInput")
    w24_d = nc.dram_tensor("w24", [128, 8], FP8, kind="ExternalInput")
    ew2b_d = nc.dram_tensor("ew2b", [65, 64], f32, kind="ExternalInput")
    ow1_d = nc.dram_tensor("ow1", [64, 256], f32, kind="ExternalInput")
    ob1_d = nc.dram_tensor("ob1", [1, 64], f32, kind="ExternalInput")
    w2o_d = nc.dram_tensor("w2o", [65, 1], f32, kind="ExternalInput")
    id4_d = nc.dram_tensor("id4", [4, 4], f32, kind="ExternalInput")
    on16_d = nc.dram_tensor("on16", [1, BPC], f32, kind="ExternalInput")
    out_d = nc.dram_tensor("out", [BPC], f32, kind="ExternalOutput")

    with tile.TileContext(nc) as tc, ExitStack() as ctx:
        consts = ctx.enter_context(tc.tile_pool(name="consts", bufs=1))

        def cload(dram, shape, nm, dt=f32):
            t = consts.tile(shape, dt, name=nm, tag=nm)
            nc.sync.dma_start(t[:], dram[:])
            return t

        wsc = cload(wsc_d, [128, PAIRS * 512], "wsc", FP8)
        wenm = cload(wenm_d, [128, PAIRS * 130], "wenm", FP8)
        w24 = cload(w24_d, [128, 8], "w24", FP8)
        ew2b = cload(ew2b_d, [65, 64], "ew2b")
        ow1 = cload(ow1_d, [64, 256], "ow1")
        ob1 = cload(ob1_d, [1, 64], "ob1")
        w2o = cload(w2o_d, [65, 1], "w2o")
        id4 = cload(id4_d, [4, 4], "id4")
        on16 = cload(on16_d, [1, BPC], "on16")

        ctxnT = consts.tile([65, 64], f32, name="ctxnT", tag="ctxnT")
        obuf = consts.tile([65, BPC], f32, name="obuf", tag="obuf")
        fct = consts.tile([64, 64], f32, name="fct", tag="fct")
        res = consts.tile([1, BPC], f32, name="res", tag="res")
        nc.vector.memset(ctxnT[64:65, :], 1.0)
        nc.vector.memset(obuf[64:65, :], 1.0)

        with ExitStack() as pctx:
            xapool = pctx.enter_context(tc.tile_pool(name="xap", bufs=3))
            shpool = pctx.enter_context(tc.tile_pool(name="shp", bufs=18))
            h1pool = pctx.enter_context(tc.tile_pool(name="h1p", bufs=2))
            enmpool = pctx.enter_context(tc.tile_pool(name="enmp", bufs=2))
            smpool = pctx.enter_context(tc.tile_pool(name="smp", bufs=2))
            psA = pctx.enter_context(tc.tile_pool(name="psA", bufs=1, space="PSUM"))
            psB = pctx.enter_context(tc.tile_pool(name="psB", bufs=1, space="PSUM"))
            psLG = pctx.enter_context(tc.tile_pool(name="psLG", bufs=1, space="PSUM"))
            psH1 = pctx.enter_context(tc.tile_pool(name="psH1", bufs=1, space="PSUM"))
            psCX = pctx.enter_context(tc.tile_pool(name="psCX", bufs=1, space="PSUM"))

            xas = {}

            def load_xa(p):
                t = xapool.tile([128, N], FP8, name="xa", tag="xa")
                nc.sync.dma_start(t[:], xpa_d[p])
                xas[p] = t

            load_xa(0)
            if PAIRS > 1:
                load_xa(1)

            def pool_mms(cx, enm, h1a, ts):
                for t in ts:
                    for i in (0, 1):
                        off = 260 * (NSTR * i + t // 4) + 65 * (t % 4)
                        nc.tensor.matmul(
                            cx[0:4, 65 * i : 65 * i + 65],
                            enm[:, 256 * i + 4 * t : 256 * i + 4 * t + 4],
                            h1a[:, off : off + 65],
                            start=(t == 0), stop=(t == NB - 1),
                            skip_group_check=True,
                        )

            def epilogue(cx, p):
                rz = smpool.tile([4, 2], f32, name="rz", tag="rz")
                nc.vector.reciprocal(rz[:, 0:1], cx[0:4, 64:65])
                nc.vector.reciprocal(rz[:, 1:2], cx[0:4, 129:130])
                cpn = smpool.tile([4, 128], f32, name="cpn", tag="cpn")
                nc.vector.tensor_scalar_mul(cpn[:, 0:64], cx[0:4, 0:64], rz[:, 0:1])
                nc.vector.tensor_scalar_mul(cpn[:, 64:128], cx[0:4, 65:129], rz[:, 1:2])
                tp = cx[:, 130:134]
                nc.tensor.transpose(tp[:], cpn[:], id4[:])
                nc.vector.tensor_copy(
                    out=ctxnT[0:64, 8 * p : 8 * p + 4], in_=tp[0:64, :]
                )
                nc.vector.tensor_copy(
                    out=ctxnT[0:64, 8 * p + 4 : 8 * p + 8], in_=tp[64:128, :]
                )

            # Software-pipelined across pairs: in super-iteration q,
            # pair q+1 runs its score matmuls + relus (sh_sb buffered in
            # SBUF for a full pair), pair q runs logits + h1, pair q-1 runs
            # pooling. Removes the relu -> logit latency from the chunk
            # critical path entirely.
            shs = {}   # (pair, chunk) -> relu'd sh_sb tile
            lgs = {}
            h1as = {}
            cxs = {}

            def sh_step(p, cc):
                cs = slice(512 * cc, 512 * (cc + 1))
                xa = xas[p]
                sAB = psA.tile([128, 1024], f32, name="sAB", tag="sAB")
                sCD = psB.tile([128, 1024], f32, name="sCD", tag="sCD")
                order = (0, 1, 2, 3) if cc % 2 == 0 else (2, 3, 0, 1)
                for s in order:
                    dst = (sAB if s < 2 else sCD)[:, 512 * (s % 2) : 512 * (s % 2) + 512]
                    nc.tensor.matmul(
                        dst,
                        wsc[:, 512 * p + 128 * s : 512 * p + 128 * s + 128],
                        xa[:, cs],
                        start=True, stop=True,
                        skip_group_check=True,
                    )
                sh_sb = shpool.tile([128, 2048], FP8, name="sh_sb", tag="shs")
                if cc % 2 == 0:
                    nc.vector.tensor_scalar(sh_sb[:, 0:1024], sAB[:], 0.0, None, ALU.max)
                    nc.scalar.activation(sh_sb[:, 1024:2048], sCD[:], AF.Relu)
                else:
                    nc.scalar.activation(sh_sb[:, 1024:2048], sCD[:], AF.Relu)
                    nc.vector.tensor_scalar(sh_sb[:, 0:1024], sAB[:], 0.0, None, ALU.max)
                shs[(p, cc)] = sh_sb

            def lgh1_step(p, cc):
                sh_sb = shs.pop((p, cc))
                xa = xas[p]
                lg = lgs[p]
                h1a = h1as[p]
                for i in (0, 1):
                    for j in range(4):
                        t = 4 * cc + j
                        lgo = lg[:, 256 * i + 4 * t : 256 * i + 4 * t + 4]
                        base = 1024 * i
                        nc.tensor.matmul(
                            lgo,
                            sh_sb[:, base + 128 * j : base + 128 * j + 128],
                            w24[:, 0:4],
                            start=True, stop=False,
                            skip_group_check=True,
                        )
                        nc.tensor.matmul(
                            lgo,
                            sh_sb[:, base + 512 + 128 * j : base + 512 + 128 * j + 128],
                            w24[:, 4:8],
                            start=False, stop=True,
                            skip_group_check=True,
                        )
                h1A = psH1.tile([128, 260], f32, name="h1A", tag="h1A")
                h1B = psH1.tile([128, 260], f32, name="h1B", tag="h1B")
                for j in range(4):
                    t = 4 * cc + j
                    for ht, o in ((h1A, 0), (h1B, 65)):
                        nc.tensor.matmul(
                            ht[:, 65 * j : 65 * j + 65],
                            xa[:, 128 * t : 128 * t + 128],
                            wenm[:, 130 * p + o : 130 * p + o + 65],
                            start=True, stop=True,
                            skip_group_check=True,
                        )
                hsA = h1a[:, 260 * cc : 260 * (cc + 1)]
                hsB = h1a[:, 260 * (NSTR + cc) : 260 * (NSTR + cc + 1)]
                if cc % 2 == 0:
                    nc.vector.tensor_scalar(hsA, h1A[:], 0.0, None, ALU.max)
                    nc.scalar.activation(hsB, h1B[:], AF.Relu)
                else:
                    nc.scalar.activation(hsA, h1A[:], AF.Relu)
                    nc.vector.tensor_scalar(hsB, h1B[:], 0.0, None, ALU.max)

            def open_pair(p):
                lgs[p] = psLG.tile([128, 512], f32, name="lg", tag="lg")
                h1as[p] = h1pool.tile([128, 2 * NSTR * 260], BF16, name="h1a", tag="h1a")

            def close_pair(p):
                # exp -> enm; pooling of p happens in super-iteration p+1
                enm = enmpool.tile([128, 512], BF16, name="enm", tag="enm")
                nc.scalar.activation(enm[:], lgs.pop(p)[:], AF.Exp)
                return (enm, h1as.pop(p))

            prevs = {}
            open_pair(0)
            for cc in range(NCHB):
                sh_step(0, cc)
            for q in range(PAIRS):
                # super-iteration q: sh for q+1, logits/h1 for q, pool for q-1
                if q + 1 < PAIRS:
                    open_pair(q + 1)
                if q + 2 < PAIRS:
                    load_xa(q + 2)
                if q - 1 >= 0:
                    cxs[q - 1] = psCX.tile([128, 134], f32, name="cx", tag="cx")
                for cc in range(NCHB):
                    if q + 1 < PAIRS:
                        sh_step(q + 1, cc)
                    lgh1_step(q, cc)
                    if q - 1 >= 0:
                        pool_mms(cxs[q - 1], prevs[q - 1][0], prevs[q - 1][1],
                                 (4 * cc, 4 * cc + 1, 4 * cc + 2, 4 * cc + 3))
                if q - 1 >= 0:
                    epilogue(cxs.pop(q - 1), q - 1)
                    del prevs[q - 1]
                prevs[q] = close_pair(q)
                if q - 1 >= 0:
                    xas.pop(q - 1)

            cxp = psCX.tile([128, 134], f32, name="cx", tag="cx")
            for cc in range(NCHB):
                pool_mms(cxp, prevs[PAIRS - 1][0], prevs[PAIRS - 1][1],
                         (4 * cc, 4 * cc + 1, 4 * cc + 2, 4 * cc + 3))
            epilogue(cxp, PAIRS - 1)

        # ---- Phase D: pooled-context encoder layer 2 + output MLP
        with ExitStack() as pctx:
            psD = pctx.enter_context(tc.tile_pool(name="psD", bufs=1, space="PSUM"))
            fct_ps = psD.tile([64, 64], f32, name="fct_ps", tag="fctp")
            nc.tensor.matmul(fct_ps[:], ew2b[:], ctxnT[:], start=True, stop=True)
            nc.vector.tensor_copy(out=fct[:], in_=fct_ps[:])
            fct_bh = fct.rearrange("d (b h) -> d b h", h=HEADS)
            o1_ps = psD.tile([64, BPC], f32, name="o1_ps", tag="o1p")
            for h in range(HEADS):
                nc.tensor.matmul(
                    o1_ps[:],
                    ow1[:, h * 64 : (h + 1) * 64],
                    fct_bh[:, :, h],
                    start=(h == 0),
                    stop=False,
                    skip_group_check=True,
                )
            nc.tensor.matmul(
                o1_ps[:], ob1[:], on16[:], start=False, stop=True,
                skip_group_check=True,
            )
            nc.scalar.activation(obuf[0:64, :], o1_ps[:], AF.Relu)
            fin_ps = psD.tile([1, BPC], f32, name="fin_ps", tag="finp")
            nc.tensor.matmul(fin_ps[:], w2o[:], obuf[:], start=True, stop=True)
            nc.vector.tensor_copy(out=res[:], in_=fin_ps[:])
            nc.sync.dma_start(out_d.rearrange("(a n) -> a n", a=1), res[:])

    if not nc.is_finalized():
        nc.finalize()
    return nc


def make_in_maps(inputs):
    """Host-side marshalling: fp8 hi/lo packing; radar folded into biases;
    point rows replicated across all 128 partitions (8 copies per b)."""
    import ml_dtypes

    f8 = ml_dtypes.float8_e4m3fn
    f = np.float32

    def split8(a):
        hi = a.astype(f8)
        lo = (a - hi.astype(f)).astype(f8)
        return hi, lo

    radar = np.concatenate(
        [np.asarray(inputs["radar_xy"], f), np.asarray(inputs["radar_dir"], f)], axis=1
    )
    pts = np.asarray(inputs["pts"], f)
    enc_w1 = np.asarray(inputs["enc_w1"], f)
    enc_b1 = np.asarray(inputs["enc_b1"], f)
    enc_w2 = np.asarray(inputs["enc_w2"], f)
    enc_b2 = np.asarray(inputs["enc_b2"], f)
    sc_w1 = np.asarray(inputs["sc_w1"], f)
    sc_b1 = np.asarray(inputs["sc_b1"], f)
    sc_w2 = np.asarray(inputs["sc_w2"], f)
    out_w1 = np.asarray(inputs["out_w1"], f)
    out_b1 = np.asarray(inputs["out_b1"], f)
    out_w2 = np.asarray(inputs["out_w2"], f)
    out_b2 = np.asarray(inputs["out_b2"], f)

    cb_sc = np.einsum("br,hrd->bhd", radar, sc_w1[:, :4, :]) + sc_b1  # [B, 4, 64]
    cb_enc = radar @ enc_w1[:4] + enc_b1  # [B, 64]

    x = pts[:, :, 0]
    y = pts[:, :, 1]
    xh, xl = split8(x)
    yh, yl = split8(y)

    # 8 feature rows per b: [xh, yh, xh, yh, xl, yl, 1, 1]
    xrows = np.empty((B, 8, N), f8)
    xrows[:, 0] = xh
    xrows[:, 1] = yh
    xrows[:, 2] = xh
    xrows[:, 3] = yh
    xrows[:, 4] = xl
    xrows[:, 5] = yl
    xrows[:, 6] = 1.0
    xrows[:, 7] = 1.0

    # score stationary rows: [wxh, wyh, wxl, wyl, wxh, wyh, cbh, cbl]
    wx = sc_w1[:, 4, :]
    wy = sc_w1[:, 5, :]
    wxh_, wxl_ = split8(wx)
    wyh_, wyl_ = split8(wy)

    def sc_stat(b, half, row0):
        st = np.zeros((128, 128), f8)
        for hh in range(2):
            h = half * 2 + hh
            s = slice(hh * 64, hh * 64 + 64)
            st[row0 + 0, s] = wxh_[h]
            st[row0 + 1, s] = wyh_[h]
            st[row0 + 2, s] = wxl_[h]
            st[row0 + 3, s] = wyl_[h]
            st[row0 + 4, s] = wxh_[h]
            st[row0 + 5, s] = wyh_[h]
            cbh, cbl = split8(cb_sc[b, h])
            st[row0 + 6, s] = cbh
            st[row0 + 7, s] = cbl
        return st

    exh_, exl_ = split8(enc_w1[4])
    eyh_, eyl_ = split8(enc_w1[5])

    def enc_rhs(b, row0):
        st = np.zeros((128, 65), f8)
        st[row0 + 0, :64] = exh_
        st[row0 + 1, :64] = eyh_
        st[row0 + 2, :64] = exl_
        st[row0 + 3, :64] = eyl_
        st[row0 + 4, :64] = exh_
        st[row0 + 5, :64] = eyh_
        cbh, cbl = split8(cb_enc[b])
        st[row0 + 6, :64] = cbh
        st[row0 + 7, :64] = cbl
        st[row0 + 6, 64] = 1.0
        return st

    w24 = np.zeros((128, 8), f8)
    w24[0:64, 0] = sc_w2[0].astype(f8)
    w24[64:128, 1] = sc_w2[1].astype(f8)
    w24[0:64, 6] = sc_w2[2].astype(f8)
    w24[64:128, 7] = sc_w2[3].astype(f8)

    ew2b = np.concatenate([enc_w2, enc_b2[None, :]], axis=0)
    ow1 = np.empty((64, 256), f)
    for h in range(HEADS):
        ow1[:, h * 64 : (h + 1) * 64] = out_w1[h * 64 : (h + 1) * 64, :]
    ob1 = np.ascontiguousarray(out_b1[None, :])
    w2o = np.concatenate([out_w2, out_b2[None, :]], axis=0)
    id4 = np.eye(4, dtype=f)
    on16 = np.ones((1, BPC), f)

    in_maps = []
    for core in range(NCORES):
        b0 = core * BPC
        xpa = np.empty((PAIRS, 128, N), f8)
        wsc = np.empty((128, PAIRS * 512), f8)
        wenm = np.empty((128, PAIRS * 130), f8)
        for p in range(PAIRS):
            be, bo = b0 + 2 * p, b0 + 2 * p + 1
            xpa[p, 0:64] = np.tile(xrows[be], (8, 1))
            xpa[p, 64:128] = np.tile(xrows[bo], (8, 1))
            wsc[:, 512 * p + 0 : 512 * p + 128] = sc_stat(be, 0, 0)
            wsc[:, 512 * p + 128 : 512 * p + 256] = sc_stat(be, 1, 0)
            wsc[:, 512 * p + 256 : 512 * p + 384] = sc_stat(bo, 0, 64)
            wsc[:, 512 * p + 384 : 512 * p + 512] = sc_stat(bo, 1, 64)
            wenm[:, 130 * p : 130 * p + 65] = enc_rhs(be, 0)
            wenm[:, 130 * p + 65 : 130 * p + 130] = enc_rhs(bo, 64)
        in_maps.append(
            dict(
                xpa=xpa, wsc=wsc, wenm=wenm, w24=w24, ew2b=ew2b, ow1=ow1,
                ob1=ob1, w2o=w2o, id4=id4, on16=on16,
            )
        )
    return in_maps


_CACHE = {}


def _get_runner():
    if "runner" in _CACHE:
        return _CACHE["runner"]

    import jax
    from jax.sharding import Mesh, NamedSharding, PartitionSpec

    from concourse.bass2jax import (
        _bass_exec_p,
        install_neuronx_cc_hook,
        partition_id_tensor,
        shard_map,
    )

    nc = build_nc()
    _CACHE["nc"] = nc
    install_neuronx_cc_hook()
    partition_name = nc.partition_id_tensor.name if nc.partition_id_tensor else None
    in_names, out_names, out_avals = [], [], []
    for alloc in nc.m.functions[0].allocations:
        if not isinstance(alloc, mybir.MemoryLocationSet):
            continue
        name = alloc.memorylocations[0].name
        if alloc.kind == "ExternalInput":
            if name != partition_name:
                in_names.append(name)
        elif alloc.kind == "ExternalOutput":
            out_names.append(name)
            out_avals.append(
                jax.core.ShapedArray(tuple(alloc.tensor_shape), mybir.dt.np(alloc.dtype))
            )
    all_in_names = tuple(in_names + out_names)
    if partition_name is not None:
        all_in_names = all_in_names + (partition_name,)

    def _body(*args):
        operands = list(args)
        if partition_name is not None:
            operands.append(partition_id_tensor())
        return tuple(
            _bass_exec_p.bind(
                *operands,
                out_avals=tuple(out_avals),
                in_names=all_in_names,
                out_names=tuple(out_names),
                lowering_input_output_aliases=(),
                sim_require_finite=True,
                sim_require_nnan=True,
                nc=nc,
            )
        )

    devices = jax.devices()[:NCORES]
    mesh = Mesh(np.asarray(devices), ("core",))
    nin = len(in_names) + len(out_names)
    fn = jax.jit(
        shard_map(
            _body,
            mesh=mesh,
            in_specs=(PartitionSpec("core"),) * nin,
            out_specs=(PartitionSpec("core"),) * len(out_names),
            check_rep=False,
        ),
        keep_unused=True,
    )
    sharding = NamedSharding(mesh, PartitionSpec("core"))
    runner = (fn, sharding, in_names, out_avals)
    _CACHE["runner"] = runner
    return runner


def kernel(**inputs):
    import jax

    in_maps = make_in_maps(inputs)
    fn, sharding, in_names, out_avals = _get_runner()
    concat_in = [
        np.concatenate([np.asarray(in_maps[c][name]) for c in range(NCORES)], axis=0)
        for name in in_names
    ]
    concat_zeros = [
        np.zeros((NCORES * a.shape[0], *a.shape[1:]), a.dtype) for a in out_avals
    ]
    args = [jax.device_put(a, sharding) for a in (*concat_in, *concat_zeros)]
    (out,) = fn(*args)
    return np.asarray(out).reshape(B).astype(np.float32)


# revision 6
# speedup vs baseline: 1.0996x; 1.0043x over previous
"""Trainium2 Bass kernel for nn_MultiHeadMLPAttentionModel — K=128-dense design.

All matmuls use full-K=128 operands (zero-padded stationaries / zero-padded
weight rhs) so the PE's HAM activity monitor sees dense streams and holds the
clock at 2.4 GHz for the whole kernel (K=8 row-tiled matmuls read as "idle"
and get throttled to 1.2 GHz — measured).

Per core: 16 batch rows as 8 b-pairs. Point data is replicated across all
128 SBUF partitions (8 copies of each b's 8 feature rows); stationaries are
zero except the 8 rows matching their b's home slot, so K=128 contraction
reproduces the K=8 result at identical stream cost.

Per pair (16 chunks of 512 points):
  * 4 serial K=128 fp8 matmuls -> score hiddens for both b's (bias folded
    into contraction rows; x and layer-1 weights split hi/lo for precision),
    into two [128,1024] 2-bank PSUM tiles; one [128,1024] relu op each
    (vector / scalar) -> fp8 SBUF.
  * logits point-major: 16 tiny N=4 matmuls per chunk, lhsT = sh slices,
    rhs = zero-padded w2 columns; all 64 blocks of each b accumulate into
    one [128,512] PSUM tile (cols 4t+h) -> exp once per pair -> bf16 enm.
  * encoder h1 point-major: lhsT = the (replicated) data block, two rhs
    (be / bo wenm columns) share each stationary; relu -> bf16 strips.
  * pooling of the PREVIOUS pair interleaved (8 matmuls per chunk) so the
    PE never idles between pairs.
Phase D (once): enc layer 2 + output MLP on pooled contexts -> [16].
"""

import numpy as np

import concourse.bass as bass
import concourse.tile as tile
from concourse import bacc, mybir

B, N, HID, HEADS = 128, 8192, 64, 4
NCORES = 8
BPC = B // NCORES      # 16 batch rows per core
PAIRS = BPC // 2       # 8 b-pairs
NCHB = N // 512        # 16 chunks per pair
NB = N // 128          # 64 point-blocks per b
NSTR = 16              # h1 strips per b (4 blocks each)

F32 = mybir.dt.float32
BF16 = mybir.dt.bfloat16
FP8 = mybir.dt.float8e4
AF = mybir.ActivationFunctionType
ALU = mybir.AluOpType


def build_nc():
    from contextlib import ExitStack

    nc = bacc.Bacc()
    f32 = F32

    xpa_d = nc.dram_tensor("xpa", [PAIRS, 128, N], FP8, kind="ExternalInput")
    wsc_d = nc.dram_tensor("wsc", [128, PAIRS * 512], FP8, kind="ExternalInput")
    wenm_d = nc.dram_tensor("wenm", [128, PAIRS * 130], FP8, kind="ExternalInput")
    w24_d = nc.dram_tensor("w24", [128, 8], FP8, kind="ExternalInput")
    ew2b_d = nc.dram_tensor("ew2b", [65, 64], f32, kind="ExternalInput")
    ow1_d = nc.dram_tensor("ow1", [64, 256], f32, kind="ExternalInput")
    ob1_d = nc.dram_tensor("ob1", [1, 64], f32, kind="ExternalInput")
    w2o_d = nc.dram_tensor("w2o", [65, 1], f32, kind="ExternalInput")
    id4_d = nc.dram_tensor("id4", [4, 4], f32, kind="ExternalInput")
    on16_d = nc.dram_tensor("on16", [1, BPC], f32, kind="ExternalInput")
    out_d = nc.dram_tensor("out", [BPC], f32, kind="ExternalOutput")

    with tile.TileContext(nc) as tc, ExitStack() as ctx:
        consts = ctx.enter_context(tc.tile_pool(name="consts", bufs=1))

        def cload(dram, shape, nm, dt=f32):
            t = consts.tile(shape, dt, name=nm, tag=nm)
            nc.sync.dma_start(t[:], dram[:])
            return t

        wsc = cload(wsc_d, [128, PAIRS * 512], "wsc", FP8)
        wenm = cload(wenm_d, [128, PAIRS * 130], "wenm", FP8)
        w24 = cload(w24_d, [128, 8], "w24", FP8)
        ew2b = cload(ew2b_d, [65, 64], "ew2b")
        ow1 = cload(ow1_d, [64, 256], "ow1")
        ob1 = cload(ob1_d, [1, 64], "ob1")
        w2o = cload(w2o_d, [65, 1], "w2o")
        id4 = cload(id4_d, [4, 4], "id4")
        on16 = cload(on16_d, [1, BPC], "on16")

        ctxnT = consts.tile([65, 64], f32, name="ctxnT", tag="ctxnT")
        obuf = consts.tile([65, BPC], f32, name="obuf", tag="obuf")
        fct = consts.tile([64, 64], f32, name="fct", tag="fct")
        res = consts.tile([1, BPC], f32, name="res", tag="res")
        nc.vector.memset(ctxnT[64:65, :], 1.0)
        nc.vector.memset(obuf[64:65, :], 1.0)

        with ExitStack() as pctx:
            xapool = pctx.enter_context(tc.tile_pool(name="xap", bufs=3))
            shpool = pctx.enter_context(tc.tile_pool(name="shp", bufs=18))
            h1pool = pctx.enter_context(tc.tile_pool(name="h1p", bufs=2))
            enmpool = pctx.enter_context(tc.tile_pool(name="enmp", bufs=2))
            smpool = pctx.enter_context(tc.tile_pool(name="smp", bufs=2))
            psA = pctx.enter_context(tc.tile_pool(name="psA", bufs=1, space="PSUM"))
            psB = pctx.enter_context(tc.tile_pool(name="psB", bufs=1, space="PSUM"))
            psLG = pctx.enter_context(tc.tile_pool(name="psLG", bufs=1, space="PSUM"))
            psH1 = pctx.enter_context(tc.tile_pool(name="psH1", bufs=1, space="PSUM"))
            psCX = pctx.enter_context(tc.tile_pool(name="psCX", bufs=1, space="PSUM"))

            xas = {}

            def load_xa(p):
                t = xapool.tile([128, N], FP8, name="xa", tag="xa")
                nc.sync.dma_start(t[:], xpa_d[p])
                xas[p] = t

            load_xa(0)
            if PAIRS > 1:
                load_xa(1)

            def pool_mms(cx, enm, h1a, ts):
                for t in ts:
                    for i in (0, 1):
                        off = 260 * (NSTR * i + t // 4) + 65 * (t % 4)
                        nc.tensor.matmul(
                            cx[0:4, 65 * i : 65 * i + 65],
                            enm[:, 256 * i + 4 * t : 256 * i + 4 * t + 4],
                            h1a[:, off : off + 65],
                            start=(t == 0), stop=(t == NB - 1),
                            skip_group_check=True,
                        )

            def epilogue(cx, p):
                rz = smpool.tile([4, 2], f32, name="rz", tag="rz")
                nc.vector.reciprocal(rz[:, 0:1], cx[0:4, 64:65])
                nc.vector.reciprocal(rz[:, 1:2], cx[0:4, 129:130])
                cpn = smpool.tile([4, 128], f32, name="cpn", tag="cpn")
                nc.vector.tensor_scalar_mul(cpn[:, 0:64], cx[0:4, 0:64], rz[:, 0:1])
                nc.vector.tensor_scalar_mul(cpn[:, 64:128], cx[0:4, 65:129], rz[:, 1:2])
                tp = cx[:, 130:134]
                nc.tensor.transpose(tp[:], cpn[:], id4[:])
                nc.vector.tensor_copy(
                    out=ctxnT[0:64, 8 * p : 8 * p + 4], in_=tp[0:64, :]
                )
                nc.vector.tensor_copy(
                    out=ctxnT[0:64, 8 * p + 4 : 8 * p + 8], in_=tp[64:128, :]
                )

            # Software-pipelined across pairs: in super-iteration q,
            # pair q+1 runs its score matmuls + relus (sh_sb buffered in
            # SBUF for a full pair), pair q runs logits + h1, pair q-1 runs
            # pooling. Removes the relu -> logit latency from the chunk
            # critical path entirely.
            shs = {}   # (pair, chunk) -> relu'd sh_sb tile
            lgs = {}
            h1as = {}
            cxs = {}

            def sh_step(p, cc):
                cs = slice(512 * cc, 512 * (cc + 1))
                xa = xas[p]
                sAB = psA.tile([128, 1024], f32, name="sAB", tag="sAB")
                sCD = psB.tile([128, 1024], f32, name="sCD", tag="sCD")
                order = (0, 1, 2, 3) if cc % 2 == 0 else (2, 3, 0, 1)
                for s in order:
                    dst = (sAB if s < 2 else sCD)[:, 512 * (s % 2) : 512 * (s % 2) + 512]
                    nc.tensor.matmul(
                        dst,
                        wsc[:, 512 * p + 128 * s : 512 * p + 128 * s + 128],
                        xa[:, cs],
                        start=True, stop=True,
                        skip_group_check=True,
                    )
                sh_sb = shpool.tile([128, 2048], FP8, name="sh_sb", tag="shs")
                if cc % 2 == 0:
                    nc.vector.tensor_scalar(sh_sb[:, 0:1024], sAB[:], 0.0, None, ALU.max)
                    nc.scalar.activation(sh_sb[:, 1024:2048], sCD[:], AF.Relu)
                else:
                    nc.scalar.activation(sh_sb[:, 1024:2048], sCD[:], AF.Relu)
                    nc.vector.tensor_scalar(sh_sb[:, 0:1024], sAB[:], 0.0, None, ALU.max)
                shs[(p, cc)] = sh_sb

            def lgh1_step(p, cc):
                sh_sb = shs.pop((p, cc))
                xa = xas[p]
                lg = lgs[p]
                h1a = h1as[p]
                for i in (0, 1):
                    for j in range(4):
                        t = 4 * cc + j
                        lgo = lg[:, 256 * i + 4 * t : 256 * i + 4 * t + 4]
                        base = 1024 * i
                        nc.tensor.matmul(
                            lgo,
                            sh_sb[:, base + 128 * j : base + 128 * j + 128],
                            w24[:, 0:4],
                            start=True, stop=False,
                            skip_group_check=True,
                        )
                        nc.tensor.matmul(
                            lgo,
                            sh_sb[:, base + 512 + 128 * j : base + 512 + 128 * j + 128],
                            w24[:, 4:8],
                            start=False, stop=True,
                            skip_group_check=True,
                        )
                h1A = psH1.tile([128, 260], f32, name="h1A", tag="h1A")
                h1B = psH1.tile([128, 260], f32, name="h1B", tag="h1B")
                for j in range(4):
                    t = 4 * cc + j
                    for ht, o in ((h1A, 0), (h1B, 65)):
                        nc.tensor.matmul(
                            ht[:, 65 * j : 65 * j + 65],
                            xa[:, 128 * t : 128 * t + 128],
                            wenm[:, 130 * p + o : 130 * p + o + 65],
                            start=True, stop=True,
                            skip_group_check=True,
                        )
                hsA = h1a[:, 260 * cc : 260 * (cc + 1)]
                hsB = h1a[:, 260 * (NSTR + cc) : 260 * (NSTR + cc + 1)]
                if cc % 2 == 0:
                    nc.vector.tensor_scalar(hsA, h1A[:], 0.0, None, ALU.max)
                    nc.scalar.activation(hsB, h1B[:], AF.Relu)
                else:
                    nc.scalar.activation(hsA, h1A[:], AF.Relu)
                    nc.vector.tensor_scalar(hsB, h1B[:], 0.0, None, ALU.max)

            def open_pair(p):
                lgs[p] = psLG.tile([128, 512], f32, name="lg", tag="lg")
                h1as[p] = h1pool.tile([128, 2 * NSTR * 260], BF16, name="h1a", tag="h1a")

            def close_pair(p):
                # exp -> enm; pooling of p happens in super-iteration p+1
                enm = enmpool.tile([128, 512], BF16, name="enm", tag="enm")
                nc.scalar.activation(enm[:], lgs.pop(p)[:], AF.Exp)
                return (enm, h1as.pop(p))

            prevs = {}
            open_pair(0)
            for cc in range(NCHB):
                sh_step(0, cc)
            for q in range(PAIRS):
                # super-iteration q: sh for q+1, logits/h1 for q, pool for q-1
                if q + 1 < PAIRS:
                    open_pair(q + 1)
                if q + 2 < PAIRS:
                    load_xa(q + 2)
                if q - 1 >= 0:
                    cxs[q - 1] = psCX.tile([128, 134], f32, name="cx", tag="cx")
                for cc in range(NCHB):
                    lgh1_step(q, cc)
                    if q + 1 < PAIRS:
                        sh_step(q + 1, cc)
                    if q - 1 >= 0:
                        pool_mms(cxs[q - 1], prevs[q - 1][0], prevs[q - 1][1],
                                 (4 * cc, 4 * cc + 1, 4 * cc + 2, 4 * cc + 3))
                if q - 1 >= 0:
                    epilogue(cxs.pop(q - 1), q - 1)
                    del prevs[q - 1]
                prevs[q] = close_pair(q)
                if q - 1 >= 0:
                    xas.pop(q - 1)

            cxp = psCX.tile([128, 134], f32, name="cx", tag="cx")
            for cc in range(NCHB):
                pool_mms(cxp, prevs[PAIRS - 1][0], prevs[PAIRS - 1][1],
                         (4 * cc, 4 * cc + 1, 4 * cc + 2, 4 * cc + 3))
            epilogue(cxp, PAIRS - 1)

        # ---- Phase D: pooled-context encoder layer 2 + output MLP
        with ExitStack() as pctx:
            psD = pctx.enter_context(tc.tile_pool(name="psD", bufs=1, space="PSUM"))
            fct_ps = psD.tile([64, 64], f32, name="fct_ps", tag="fctp")
            nc.tensor.matmul(fct_ps[:], ew2b[:], ctxnT[:], start=True, stop=True)
            nc.vector.tensor_copy(out=fct[:], in_=fct_ps[:])
            fct_bh = fct.rearrange("d (b h) -> d b h", h=HEADS)
            o1_ps = psD.tile([64, BPC], f32, name="o1_ps", tag="o1p")
            for h in range(HEADS):
                nc.tensor.matmul(
                    o1_ps[:],
                    ow1[:, h * 64 : (h + 1) * 64],
                    fct_bh[:, :, h],
                    start=(h == 0),
                    stop=False,
                    skip_group_check=True,
                )
            nc.tensor.matmul(
                o1_ps[:], ob1[:], on16[:], start=False, stop=True,
                skip_group_check=True,
            )
            nc.scalar.activation(obuf[0:64, :], o1_ps[:], AF.Relu)
            fin_ps = psD.tile([1, BPC], f32, name="fin_ps", tag="finp")
            nc.tensor.matmul(fin_ps[:], w2o[:], obuf[:], start=True, stop=True)
            nc.vector.tensor_copy(out=res[:], in_=fin_ps[:])
            nc.sync.dma_start(out_d.rearrange("(a n) -> a n", a=1), res[:])

    if not nc.is_finalized():
        nc.finalize()
    return nc


def make_in_maps(inputs):
    """Host-side marshalling: fp8 hi/lo packing; radar folded into biases;
    point rows replicated across all 128 partitions (8 copies per b)."""
    import ml_dtypes

    f8 = ml_dtypes.float8_e4m3fn
    f = np.float32

    def split8(a):
        hi = a.astype(f8)
        lo = (a - hi.astype(f)).astype(f8)
        return hi, lo

    radar = np.concatenate(
        [np.asarray(inputs["radar_xy"], f), np.asarray(inputs["radar_dir"], f)], axis=1
    )
    pts = np.asarray(inputs["pts"], f)
    enc_w1 = np.asarray(inputs["enc_w1"], f)
    enc_b1 = np.asarray(inputs["enc_b1"], f)
    enc_w2 = np.asarray(inputs["enc_w2"], f)
    enc_b2 = np.asarray(inputs["enc_b2"], f)
    sc_w1 = np.asarray(inputs["sc_w1"], f)
    sc_b1 = np.asarray(inputs["sc_b1"], f)
    sc_w2 = np.asarray(inputs["sc_w2"], f)
    out_w1 = np.asarray(inputs["out_w1"], f)
    out_b1 = np.asarray(inputs["out_b1"], f)
    out_w2 = np.asarray(inputs["out_w2"], f)
    out_b2 = np.asarray(inputs["out_b2"], f)

    cb_sc = np.einsum("br,hrd->bhd", radar, sc_w1[:, :4, :]) + sc_b1  # [B, 4, 64]
    cb_enc = radar @ enc_w1[:4] + enc_b1  # [B, 64]

    x = pts[:, :, 0]
    y = pts[:, :, 1]
    xh, xl = split8(x)
    yh, yl = split8(y)

    # 8 feature rows per b: [xh, yh, xh, yh, xl, yl, 1, 1]
    xrows = np.empty((B, 8, N), f8)
    xrows[:, 0] = xh
    xrows[:, 1] = yh
    xrows[:, 2] = xh
    xrows[:, 3] = yh
    xrows[:, 4] = xl
    xrows[:, 5] = yl
    xrows[:, 6] = 1.0
    xrows[:, 7] = 1.0

    # score stationary rows: [wxh, wyh, wxl, wyl, wxh, wyh, cbh, cbl]
    wx = sc_w1[:, 4, :]
    wy = sc_w1[:, 5, :]
    wxh_, wxl_ = split8(wx)
    wyh_, wyl_ = split8(wy)

    def sc_stat(b, half, row0):
        st = np.zeros((128, 128), f8)
        for hh in range(2):
            h = half * 2 + hh
            s = slice(hh * 64, hh * 64 + 64)
            st[row0 + 0, s] = wxh_[h]
            st[row0 + 1, s] = wyh_[h]
            st[row0 + 2, s] = wxl_[h]
            st[row0 + 3, s] = wyl_[h]
            st[row0 + 4, s] = wxh_[h]
            st[row0 + 5, s] = wyh_[h]
            cbh, cbl = split8(cb_sc[b, h])
            st[row0 + 6, s] = cbh
            st[row0 + 7, s] = cbl
        return st

    exh_, exl_ = split8(enc_w1[4])
    eyh_, eyl_ = split8(enc_w1[5])

    def enc_rhs(b, row0):
        st = np.zeros((128, 65), f8)
        st[row0 + 0, :64] = exh_
        st[row0 + 1, :64] = eyh_
        st[row0 + 2, :64] = exl_
        st[row0 + 3, :64] = eyl_
        st[row0 + 4, :64] = exh_
        st[row0 + 5, :64] = eyh_
        cbh, cbl = split8(cb_enc[b])
        st[row0 + 6, :64] = cbh
        st[row0 + 7, :64] = cbl
        st[row0 + 6, 64] = 1.0
        return st

    w24 = np.zeros((128, 8), f8)
    w24[0:64, 0] = sc_w2[0].astype(f8)
    w24[64:128, 1] = sc_w2[1].astype(f8)
    w24[0:64, 6] = sc_w2[2].astype(f8)
    w24[64:128, 7] = sc_w2[3].astype(f8)

    ew2b = np.concatenate([enc_w2, enc_b2[None, :]], axis=0)
    ow1 = np.empty((64, 256), f)
    for h in range(HEADS):
        ow1[:, h * 64 : (h + 1) * 64] = out_w1[h * 64 : (h + 1) * 64, :]
    ob1 = np.ascontiguousarray(out_b1[None, :])
    w2o = np.concatenate([out_w2, out_b2[None, :]], axis=0)
    id4 = np.eye(4, dtype=f)
    on16 = np.ones((1, BPC), f)

    in_maps = []
    for core in range(NCORES):
        b0 = core * BPC
        xpa = np.empty((PAIRS, 128, N), f8)
        wsc = np.empty((128, PAIRS * 512), f8)
        wenm = np.empty((128, PAIRS * 130), f8)
        for p in range(PAIRS):
            be, bo = b0 + 2 * p, b0 + 2 * p + 1
            xpa[p, 0:64] = np.tile(xrows[be], (8, 1))
            xpa[p, 64:128] = np.tile(xrows[bo], (8, 1))
            wsc[:, 512 * p + 0 : 512 * p + 128] = sc_stat(be, 0, 0)
            wsc[:, 512 * p + 128 : 512 * p + 256] = sc_stat(be, 1, 0)
            wsc[:, 512 * p + 256 : 512 * p + 384] = sc_stat(bo, 0, 64)
            wsc[:, 512 * p + 384 : 512 * p + 512] = sc_stat(bo, 1, 64)
            wenm[:, 130 * p : 130 * p + 65] = enc_rhs(be, 0)
            wenm[:, 130 * p + 65 : 130 * p + 130] = enc_rhs(bo, 64)
        in_maps.append(
            dict(
                xpa=xpa, wsc=wsc, wenm=wenm, w24=w24, ew2b=ew2b, ow1=ow1,
                ob1=ob1, w2o=w2o, id4=id4, on16=on16,
            )
        )
    return in_maps


_CACHE = {}


def _get_runner():
    if "runner" in _CACHE:
        return _CACHE["runner"]

    import jax
    from jax.sharding import Mesh, NamedSharding, PartitionSpec

    from concourse.bass2jax import (
        _bass_exec_p,
        install_neuronx_cc_hook,
        partition_id_tensor,
        shard_map,
    )

    nc = build_nc()
    _CACHE["nc"] = nc
    install_neuronx_cc_hook()
    partition_name = nc.partition_id_tensor.name if nc.partition_id_tensor else None
    in_names, out_names, out_avals = [], [], []
    for alloc in nc.m.functions[0].allocations:
        if not isinstance(alloc, mybir.MemoryLocationSet):
            continue
        name = alloc.memorylocations[0].name
        if alloc.kind == "ExternalInput":
            if name != partition_name:
                in_names.append(name)
        elif alloc.kind == "ExternalOutput":
            out_names.append(name)
            out_avals.append(
                jax.core.ShapedArray(tuple(alloc.tensor_shape), mybir.dt.np(alloc.dtype))
            )
    all_in_names = tuple(in_names + out_names)
    if partition_name is not None:
        all_in_names = all_in_names + (partition_name,)

    def _body(*args):
        operands = list(args)
        if partition_name is not None:
            operands.append(partition_id_tensor())
        return tuple(
            _bass_exec_p.bind(
                *operands,
                out_avals=tuple(out_avals),
                in_names=all_in_names,
                out_names=tuple(out_names),
                lowering_input_output_aliases=(),
                sim_require_finite=True,
                sim_require_nnan=True,
                nc=nc,
            )
        )

    devices = jax.devices()[:NCORES]
    mesh = Mesh(np.asarray(devices), ("core",))
    nin = len(in_names) + len(out_names)
    fn = jax.jit(
        shard_map(
            _body,
            mesh=mesh,
            in_specs=(PartitionSpec("core"),) * nin,
            out_specs=(PartitionSpec("core"),) * len(out_names),
            check_rep=False,
        ),
        keep_unused=True,
    )
    sharding = NamedSharding(mesh, PartitionSpec("core"))
    runner = (fn, sharding, in_names, out_avals)
    _CACHE["runner"] = runner
    return runner


def kernel(**inputs):
    import jax

    in_maps = make_in_maps(inputs)
    fn, sharding, in_names, out_avals = _get_runner()
    concat_in = [
        np.concatenate([np.asarray(in_maps[c][name]) for c in range(NCORES)], axis=0)
        for name in in_names
    ]
    concat_zeros = [
        np.zeros((NCORES * a.shape[0], *a.shape[1:]), a.dtype) for a in out_avals
    ]
    args = [jax.device_put(a, sharding) for a in (*concat_in, *concat_zeros)]
    (out,) = fn(*args)
    return np.asarray(out).reshape(B).astype(np.float32)
